# revision 26
# baseline (speedup 1.0000x reference)
"""Trainium2 Bass kernel for the vq_codebook problem.

  dist_sq[n,k] = sum_d (x[n,d]-ctrs[k,d])^2 * s[d]
  out = softmax(-dist_sq, axis=1) @ values

Sharding: data-parallel over N (8192 rows of x per core); codebooks
replicated on all 8 cores. No collectives (forward only).

Math trick: softmax is shift-invariant, so
  softmax(-dist_sq)[n,k] = softmax(2*cross_s[n,k] - c_sq[k])  with
  cross_s = (x*s) @ ctrs.T,  c_sq[k] = sum_d s[d]*ctrs[k,d]^2.
We compute E = exp(2*(cross_s - 0.5*c_sq)) unnormalized (range-checked:
max exponent ~48 < 88, row-max min ~ -27, so fp32 exp never overflows
and denominators stay normal), then
  y[n,:] = (E.T @ vals_aug)[n,:256] / (E.T @ vals_aug)[n,256]
with vals_aug = [values | ones] so the denominator comes from the same
accumulating matmul.

Host-side prep (per core): the stationary operands are assembled on the
host so the device runs a dense matmul pipeline from the first cycle
(keeps the PE HAM clock-gate warm and removes the serial on-device
transpose/setup phase that kept the PE at 1.2 GHz for the first 32 us):
  xaugT [65, 8192] = [x_shard.T ; ones]          (f32, fed to fp32r mms)
  lhs1  [65, 1024] = [(ctrs*s).T ; -0.5*c_sq]    (f32)
  vals  [1024, 258] = [values | 1 | 1]           (bf16)
Phase 1 runs transposed (k on partitions, n on free): one fp32r matmul
per 128-centroid chunk produces the whole softmax argument. Phase 2 uses
E chunks (bf16, written by the exp activation) as the stationary operand
against vals, producing y in natural [n, d_out] layout.
"""

import os

os.environ.setdefault("JAX_PLATFORMS", "axon")

import numpy as np

N, D_IN, K, D_OUT = 65536, 64, 1024, 256
NCORES = 8
NS = N // NCORES  # 8192 rows per core
TROWS = 512  # rows of x per tile
NTILES = NS // TROWS  # 16
KC = K // 128  # 8 centroid chunks
NSUB = TROWS // 128  # 4 output sub-tiles per tile
CDIM = D_IN + 1  # contraction rows: 64 data dims + 1 bias row

USE_F32R = True
WARMUP_MMS = 8  # dummy matmuls spanning the DMA lead-in to keep the PE
# HAM clock-gate busy from t~0 so real compute starts at 2.4 GHz

_cache = {}


def _build(use_f32r, rows=NS, warmup=WARMUP_MMS):
    import concourse.bacc as bacc
    import concourse.tile as tile
    from concourse import mybir

    f32 = mybir.dt.float32
    mmdt = mybir.dt.float32r if use_f32r else f32
    p2dt = mybir.dt.bfloat16
    Exp = mybir.ActivationFunctionType.Exp

    ntiles = rows // TROWS
    nc = bacc.Bacc("TRN2", target_bir_lowering=False, debug=False)
    dma_start = nc.sync.dma_start
    xaugT = nc.declare_dram_parameter("xaugT", [CDIM, rows], mmdt, isOutput=False)
    lhs1_d = nc.declare_dram_parameter("lhs1", [CDIM, K], mmdt, isOutput=False)
    vals_d = nc.declare_dram_parameter("vals", [K, D_OUT + 2], p2dt, isOutput=False)
    y = nc.declare_dram_parameter("y", [rows, D_OUT], f32, isOutput=True)

    with tile.TileContext(nc) as tc:
        with (
            tc.tile_pool(name="const", bufs=1) as constp,
            tc.tile_pool(name="xsT", bufs=5) as xsTp,
            tc.tile_pool(name="E", bufs=3) as Ep,
            tc.tile_pool(name="ysb", bufs=3) as yp,
            tc.tile_pool(name="rcp", bufs=8) as rcpp,
            tc.tile_pool(name="psA", bufs=3, space="PSUM") as psA,
            tc.tile_pool(name="psO", bufs=2, space="PSUM") as psO,
        ):
            TILES = [(n0, TROWS) for n0 in range(0, rows, TROWS)]

            # tile-0 x DMA first: it is the head of the critical path
            def phase1_load(t):
                n0, tr = t
                xsT = xsTp.tile([CDIM, tr], mmdt)
                dma_start(xsT[:], xaugT[:, n0 : n0 + tr])
                return xsT

            PREFETCH = 4
            xsT_pre = [phase1_load(TILES[0])]

            # HAM warm-up: the PE would otherwise idle until the first
            # input DMA lands (~11 us: ring init + transfer), and the
            # clock-gate only reaches 2.4 GHz after a sustained busy
            # window. Chew through cheap dummy matmuls (SBUF garbage ->
            # scratch PSUM, never read) so the PE is busy and warm the
            # moment real operands arrive.
            # full-width (128-partition) operands: the HAM watches
            # array activity, so narrow dummies would not register
            wlhs = constp.tile([128, 128], p2dt)
            wrhs = constp.tile([128, TROWS], p2dt)
            nc.vector.memset(wlhs[:], 1.0)
            nc.vector.memset(wrhs[:], 0.001)
            wps = psO.tile([128, TROWS], f32, tag="psO")

            def bridge(n):
                # dummy matmuls into the same scratch PSUM tile: pure
                # PE-array activity with no cross-engine dependencies
                for _ in range(n):
                    nc.tensor.matmul(wps[:], wlhs[:], wrhs[:])

            if warmup:
                bridge(warmup)

            # constants: split lhs1 per chunk-pair so the first phase-1
            # matmul only waits on its own 66 KB slice, not all 266 KB
            lhs1 = constp.tile([CDIM, KC, 128], mmdt)
            for c in range(0, KC, 2):
                dma_start(
                    lhs1[:, c : c + 2, :],
                    lhs1_d[:, c * 128 : (c + 2) * 128].rearrange(
                        "d (c k) -> d c k", k=128
                    ),
                )
            # wire-order matters at the head: tile 0-2 x slices and the
            # first vals half must all land just-in-time for the pipeline
            # fill, so interleave the prefetches with the vals halves
            vals = constp.tile([128, KC, D_OUT + 2], p2dt)

            def vals_load(h):
                dma_start(
                    vals[:, h * (KC // 2) : (h + 1) * (KC // 2), :],
                    vals_d[h * (K // 2) : (h + 1) * (K // 2), :].rearrange(
                        "(c p) v -> p c v", p=128
                    ),
                )

            xsT_pre.append(phase1_load(TILES[1]))
            xsT_pre.append(phase1_load(TILES[2]))
            vals_load(0)
            vals_load(1)
            xsT_pre.append(phase1_load(TILES[3]))

            def p1_pair(E, xsT, c, tr):
                pe = psA.tile([128, 2, tr], f32, tag="psA")
                nc.tensor.matmul(pe[:, 0, :], lhs1[:, c, :], xsT[:])
                nc.tensor.matmul(pe[:, 1, :], lhs1[:, c + 1, :], xsT[:])
                nc.scalar.activation(E[:, c : c + 2, :], pe[:], Exp, scale=2.0)

            def p2_group(t, E, a):
                n0, tr = t
                po = psO.tile([128, D_OUT + 2], f32, tag="psO")
                for c in range(KC):
                    nc.tensor.matmul(
                        po[:],
                        E[:, c, a * 128 : (a + 1) * 128],
                        vals[:, c, :],
                        start=(c == 0),
                        stop=(c == KC - 1),
                    )
                rcp = rcpp.tile([128, 1], f32)
                nc.vector.reciprocal(rcp[:], po[:, D_OUT : D_OUT + 1])
                ysb = yp.tile([128, D_OUT], f32)
                nc.vector.tensor_scalar_mul(ysb[:], po[:, 0:D_OUT], rcp[:])
                # per-sub-tile store: lets the final tile's output drain
                # while its remaining sub-tiles still compute
                dma_start(y[n0 + a * 128 : n0 + (a + 1) * 128, :], ysb[:])

            # Interleaved emission: alternate a phase-1 chunk-pair of
            # tile i with a phase-2 group of tile i-1, so the slow fp32r
            # weight loads hide under the long bf16 p2 matmul stream.
            Eprev = None
            for i, t in enumerate(TILES):
                xsT = xsT_pre[i] if i < PREFETCH else phase1_load(t)
                E = Ep.tile([128, KC, t[1]], p2dt)
                for c in range(0, KC, 2):
                    p1_pair(E, xsT, c, t[1])
                    if i < 2:
                        # head fill: tile 0/1 phase-1 is paced by the
                        # exp chain (psA ring) and tile-0 phase-2 can't
                        # start until exp(0) completes — keep the array
                        # active so the HAM clock-gate never drops back
                        bridge(3)
                    if Eprev is not None:
                        p2_group(TILES[i - 1], Eprev, c // 2)
                Eprev = E
            for a in range(NSUB):
                p2_group(TILES[-1], Eprev, a)

    nc.compile()
    nc.finalize()
    return nc


def get_nc(use_f32r=USE_F32R, rows=NS):
    key = ("nc", use_f32r, rows)
    if key not in _cache:
        _cache[key] = _build(use_f32r, rows)
    return _cache[key]


def make_in_maps(x, ctrs, values, s):
    from concourse import mybir

    bf16 = mybir.dt.np(mybir.dt.bfloat16)
    x = np.ascontiguousarray(x, dtype=np.float32)
    ctrs = np.asarray(ctrs, dtype=np.float32)
    values = np.asarray(values, dtype=np.float32)
    s = np.asarray(s, dtype=np.float32)

    c_sq = (ctrs * ctrs) @ s  # (K,)
    lhs1 = np.empty((CDIM, K), dtype=np.float32)
    lhs1[0:D_IN] = (ctrs * s).T
    lhs1[D_IN] = -0.5 * c_sq
    vals_aug = np.ones((K, D_OUT + 2), dtype=np.float32)
    vals_aug[:, 0:D_OUT] = values
    vals_aug = vals_aug.astype(bf16)

    maps = []
    for i in range(NCORES):
        xaug = np.empty((CDIM, NS), dtype=np.float32)
        xaug[0:D_IN] = x[i * NS : (i + 1) * NS].T
        xaug[D_IN] = 1.0
        maps.append({"xaugT": xaug, "lhs1": lhs1, "vals": vals_aug})
    return maps


def run(x, ctrs, values, s, trace=False, use_f32r=USE_F32R, tmpdir=None):
    from concourse.bass_utils import run_bass_kernel_spmd

    nc = get_nc(use_f32r)
    res = run_bass_kernel_spmd(
        nc,
        make_in_maps(x, ctrs, values, s),
        list(range(NCORES)),
        trace=trace,
        tmpdir=tmpdir,
    )
    out = np.concatenate([res.results[i]["y"] for i in range(NCORES)], axis=0)
    return out, res


def kernel(x, ctrs, values, s):
    out, _ = run(x, ctrs, values, s, trace=False)
    return out.astype(np.float32)
